# revision 18
# baseline (speedup 1.0000x reference)
"""CTConv2d Trainium2 kernel.

Computes y = conv2d(x, w) where w (O,I,3,3) is synthesized on host from
core/periphery/threshold/scale (tiny tensors), and the conv runs on 8
NeuronCores, data-parallel over batch (32 images -> 4 per core).

Device kernel (per core): hybrid PE/DVE decomposition of the 3x3 conv.
All 9 taps share the same channel-mix matrix up to a scalar (the
synthesized weight is w[o,i,kh,kw] = gate-scaled core x periphery), so
the vertical tap pair (-1,0)/(+1,0) and horizontal pair (0,-1)/(0,+1)
are each collapsed on the Vector engine into one fused op
    z = (x_shifted_a * ratio) + x_shifted_b
with the pair's base coefficient folded into that pair's matmul matrix.
Per 4-row output block the PE then runs 7 accumulating fp16 matmuls
(center + 4 corner taps + z_vert + z_horiz, K=128 channels, N=448)
into fp32 PSUM instead of 9 -> ~22% less PE time, the kernel's
bottleneck. Ratios are runtime data (per-partition scalars), so the
compiled NEFF stays valid for any input values.

Layout: host bakes x into a zero-padded (114 x 116) fp16 image per
channel (interior at row 1, col 2; stride 116 keeps dw=0 reads 4-byte
aligned so the vertical fused op hits the DVE 2x mode). Image loads are
contiguous row-chunks on the SP HWDGE ring; output DMAs go on the ACT
HWDGE ring. Accumulation is fp32; inputs rounded to fp16 (~3e-4
relative absmax vs the fp32 reference).
"""

import os
import sys

# The grading/bench environment may pin JAX_PLATFORMS=cpu for the jax
# reference; this kernel needs the axon/neuron PJRT backend.
if os.environ.get("JAX_PLATFORMS") == "cpu":
    del os.environ["JAX_PLATFORMS"]

for _p in ("/opt/trn_rl_repo",):
    if os.path.isdir(_p) and _p not in sys.path:
        sys.path.append(_p)

import numpy as np

import concourse.bass as bass
import concourse.mybir as mybir
from concourse import bacc
from concourse.bass_utils import run_bass_kernel_spmd
from concourse.tile import TileContext

O = 128
I = 128
B = 32
H = 112
W = 112
NCORES = 8
BPC = B // NCORES  # images per core
HP = H + 2  # padded rows (interior at row 1)
WP = W + 4  # padded cols, stride 116 (interior at col 2)
C0 = 2  # interior column offset
RB = 4  # output rows per PSUM group (N = RB*W = 448 <= 512)
NBLK = H // RB  # 28
GRP = 4  # PSUM groups / z-piece rows (16) per output DMA
ZROWS = GRP * RB  # 16 output rows per z piece
NP = NBLK // GRP  # 7 pieces per image
# image-load chunks in padded-row units (contiguous); first chunk is
# small so the first piece's fused z ops (which need padded rows
# [0, rows+2)) are ready almost immediately.
CHUNKS = [(0, 6), (6, 18), (18, 50), (50, 82), (82, HP)]
# output blocks per z piece; small early pieces so the PE's first
# z-matmuls aren't gated on big DVE ops while the pipeline fills.
PIECE_BLOCKS = [1, 1, 2, 4, 4, 4, 4, 4, 4]
# output-DMA group sizes (blocks per out tile); tapered at the end so
# the final copy+DMA chain after the last matmul is short.
OUT_GROUPS = [4, 4, 4, 4, 4, 4, 2, 1, 1]
F32 = mybir.dt.float32
F16 = mybir.dt.float16

# PE taps: 4 corners + center; then z_vert, z_horiz
PE_TAPS = [(-1, -1), (-1, 1), (0, 0), (1, -1), (1, 1)]
NW = len(PE_TAPS) + 2  # 7 weight matrices

EPS_FOLD = 1e-7


def synth_weights(core, periphery, threshold, scale):
    """Host-side weight synthesis for the hybrid decomposition.

    Returns (wmat, ratios):
      wmat (I, 7*O) fp16, lhsT layout wmat[i, t*O+o]:
        t=0..4: corner/center taps from PE_TAPS,
        t=5: p_down * CG (vertical pair base, fold of x[h-1] ratio),
        t=6: p_right * CG (horizontal pair base).
      ratios (rv, rh) float: z_v = rv*x[h-1] + x[h+1],
                             z_h = rh*x[:,w-1] + x[:,w+1].
    """
    c = np.asarray(core, np.float64)[:, :, 0, 0]  # (O, I)
    thr = np.asarray(threshold, np.float64)  # (O,)
    s = float(np.asarray(scale, np.float64)[0])
    p = np.asarray(periphery, np.float64)  # (8,)
    gate = 1.0 / (1.0 + np.exp(-s * (np.abs(c) - thr[:, None])))  # (O, I)
    p_full = np.concatenate([p[:4], [1.0], p[4:]])  # (9,) taps row-major
    cg = c * gate

    def ptap(dh, dw):
        return p_full[(dh + 1) * 3 + (dw + 1)]

    w = np.empty((NW, O, I), np.float64)
    for t, (dh, dw) in enumerate(PE_TAPS):
        w[t] = c if (dh, dw) == (0, 0) else cg * ptap(dh, dw)

    def fold(p_a, p_b):
        # z = ratio*x_a + x_b, matmul matrix = p_b_clamped * CG;
        # p_b clamped away from zero so ratio*p_b == p_a exactly.
        pb = p_b if abs(p_b) >= EPS_FOLD else (EPS_FOLD if p_b >= 0 else -EPS_FOLD)
        return p_a / pb, pb

    rv, pv = fold(ptap(-1, 0), ptap(1, 0))  # a = x[h-1], b = x[h+1]
    rh, ph = fold(ptap(0, -1), ptap(0, 1))  # a = x[:,w-1], b = x[:,w+1]
    w[5] = cg * pv
    w[6] = cg * ph

    wmat = np.ascontiguousarray(w.transpose(2, 0, 1)).reshape(I, NW * O)
    return np.ascontiguousarray(wmat.astype(np.float16)), (rv, rh)


def build_nc():
    nc = bacc.Bacc(None)
    x_d = nc.dram_tensor("x", [BPC, I, HP * WP], F16, kind="ExternalInput")
    w_d = nc.dram_tensor("w", [I, NW * O], F16, kind="ExternalInput")
    r_d = nc.dram_tensor("r", [128, 2], F32, kind="ExternalInput")
    y_d = nc.dram_tensor("y", [BPC, O, H, W], F32, kind="ExternalOutput")

    mult = mybir.AluOpType.mult
    add = mybir.AluOpType.add

    with TileContext(nc) as tc, tc.tile_pool(name="persist", bufs=1) as persist:
        wt = persist.tile([I, NW * O], F16, name="wt", tag="wt")
        nc.sync.dma_start(out=wt[:], in_=w_d[:])
        rt = persist.tile([128, 2], F32, name="rt", tag="rt")

        imgs = []
        for ib in range(2):
            t = persist.tile([128, HP * WP], F16, name=f"img{ib}", tag=f"img{ib}")
            imgs.append(t)

        # HAM warmup: the PE clock gate sits at 1.2 GHz until ~3.4us of
        # sustained matmul activity. A dependency-free burst right after
        # engine boot flips it to 2.4 GHz before the first real matmul
        # (which waits ~5us on the weight/chunk DMA receipt chain anyway).
        warm = persist.tile([128, 640], F16, name="warm", tag="warm")
        nc.vector.memset(warm[:], 0.0)

        def load_image(b, first=False):
            img = imgs[b % 2]
            for ci, (r0, r1) in enumerate(CHUNKS):
                # image 0's first chunks go on the ACT HWDGE ring so their
                # DMA receipt overlaps the weight load's on the SP ring.
                eng = nc.scalar if first and ci < 2 else nc.sync
                eng.dma_start(
                    out=img[:, r0 * WP : r1 * WP],
                    in_=x_d[b][:, r0 * WP : r1 * WP],
                )

        with (
            tc.tile_pool(name="psum", bufs=8, space="PSUM") as psum_pool,
            tc.tile_pool(name="outp", bufs=3) as out_pool,
            tc.tile_pool(name="zp", bufs=3) as z_pool,
        ):
            # block index -> piece start block (for z tile offsets)
            piece_start = {}
            blk0 = 0
            for nb in PIECE_BLOCKS:
                for j in range(nb):
                    piece_start[blk0 + j] = (blk0, nb)
                blk0 += nb
            assert blk0 == NBLK
            # block index -> (group start block, group size)
            group_of = {}
            blk0 = 0
            for ng in OUT_GROUPS:
                for j in range(ng):
                    group_of[blk0 + j] = (blk0, ng)
                blk0 += ng
            assert blk0 == NBLK

            for k in range(10):
                pw = psum_pool.tile([128, 512], F32, name="pw", tag="ps")
                nc.tensor.matmul(
                    out=pw[:],
                    lhsT=warm[:, 0:128],
                    rhs=warm[:, 128:640],
                    start=True,
                    stop=True,
                )
            load_image(0, first=True)
            # ratios are only needed by the DVE z ops; keep this tiny DMA
            # off the critical first-chunk path.
            nc.sync.dma_start(out=rt[:], in_=r_d[:])
            for b in range(BPC):
                if b + 1 < BPC:
                    load_image(b + 1)
                img3 = imgs[b % 2].rearrange("p (h w) -> p h w", w=WP)
                yflat = y_d[b].rearrange("o h w -> o (h w)")
                zv = zh = None
                ot = None
                for blk in range(NBLK):
                    p0, pnb = piece_start[blk]
                    if blk == p0:
                        # fused pair ops on DVE for this piece's rows
                        hz = p0 * RB  # first output row
                        nr = pnb * RB  # rows in piece
                        zv = z_pool.tile([128, nr * W], F16, name="zv", tag="zv")
                        zh = z_pool.tile([128, nr * W], F16, name="zh", tag="zh")
                        zv3 = zv.rearrange("p (h w) -> p h w", w=W)
                        zh3 = zh.rearrange("p (h w) -> p h w", w=W)
                        # padded row of output row h is h+1
                        nc.vector.scalar_tensor_tensor(
                            out=zv3[:, :, :],
                            in0=img3[:, hz : hz + nr, C0 : C0 + W],
                            scalar=rt[:, 0:1],
                            in1=img3[:, hz + 2 : hz + 2 + nr, C0 : C0 + W],
                            op0=mult,
                            op1=add,
                        )
                        nc.vector.scalar_tensor_tensor(
                            out=zh3[:, :, :],
                            in0=img3[:, hz + 1 : hz + 1 + nr, C0 - 1 : C0 - 1 + W],
                            scalar=rt[:, 1:2],
                            in1=img3[:, hz + 1 : hz + 1 + nr, C0 + 1 : C0 + 1 + W],
                            op0=mult,
                            op1=add,
                        )
                    g0, gsz = group_of[blk]
                    if blk == g0:
                        ot = out_pool.tile(
                            [128, gsz * RB * W], F32, name="ot", tag="ot"
                        )
                    h0 = blk * RB
                    ps = psum_pool.tile([128, RB * W], F32, name="ps")
                    for ti, (dh, dw) in enumerate(PE_TAPS):
                        rhs = img3[
                            :,
                            h0 + 1 + dh : h0 + 1 + dh + RB,
                            C0 + dw : C0 + dw + W,
                        ]
                        nc.tensor.matmul(
                            out=ps[:],
                            lhsT=wt[:, ti * O : (ti + 1) * O],
                            rhs=rhs,
                            start=(ti == 0),
                            stop=False,
                        )
                    zoff = (blk - p0) * RB * W
                    nc.tensor.matmul(
                        out=ps[:],
                        lhsT=wt[:, 5 * O : 6 * O],
                        rhs=zv[:, zoff : zoff + RB * W],
                        start=False,
                        stop=False,
                    )
                    nc.tensor.matmul(
                        out=ps[:],
                        lhsT=wt[:, 6 * O : 7 * O],
                        rhs=zh[:, zoff : zoff + RB * W],
                        start=False,
                        stop=True,
                    )
                    joff = (blk - g0) * RB * W
                    nc.scalar.copy(out=ot[:, joff : joff + RB * W], in_=ps[:])
                    if blk == g0 + gsz - 1:
                        n = RB * W
                        nc.scalar.dma_start(
                            out=yflat[:, g0 * n : (g0 + gsz) * n], in_=ot[:]
                        )
    nc.finalize()
    return nc


_NC_CACHE = {}


def _get_nc():
    if "nc" not in _NC_CACHE:
        _NC_CACHE["nc"] = build_nc()
    return _NC_CACHE["nc"]


def _pad_images(x):
    """(B, I, H, W) fp32 -> (B, I, HP*WP) fp16, zero halo baked in."""
    xp = np.zeros((B, I, HP, WP), np.float16)
    xp[:, :, 1 : 1 + H, C0 : C0 + W] = x.astype(np.float16)
    return np.ascontiguousarray(xp.reshape(B, I, HP * WP))


def run(inputs, trace=False, **kw):
    """Run on hardware; returns (y, BassKernelResults)."""
    x = np.asarray(inputs["x"], np.float32)
    assert x.shape == (B, I, H, W), x.shape
    wmat, (rv, rh) = synth_weights(
        inputs["core"], inputs["periphery"], inputs["threshold"], inputs["scale"]
    )
    xp = _pad_images(x)
    ratios = np.empty((128, 2), np.float32)
    ratios[:, 0] = rv
    ratios[:, 1] = rh
    nc = _get_nc()
    in_maps = [
        {"x": xp[c * BPC : (c + 1) * BPC], "w": wmat, "r": ratios}
        for c in range(NCORES)
    ]
    res = run_bass_kernel_spmd(nc, in_maps, list(range(NCORES)), trace=trace, **kw)
    y = np.concatenate([res.results[c]["y"] for c in range(NCORES)], axis=0)
    return y, res


def kernel(**inputs) -> np.ndarray:
    y, _ = run(inputs)
    return y
